# revision 5
# baseline (speedup 1.0000x reference)
"""Bass/Trainium2 kernel for nn_KazdovBlock (B=4, T=2048, D=1024, R=256), 8 cores.

Sharding: core c handles (batch b = c//2, sequence half h = c%2), 1024 tokens each.
The causal cumsums only cross the shard boundary through sums of linear
functions of x, so the carry for shard h=1 is the rank-space projection of
sum(x[first half]) — a vector-matrix chain computed on the host and fed as the
scan initial value.

On-device math (everything in feature-on-partitions / tokens-on-free layout):
  Mq = Wq.T @ [V_b | Winv.T@W_b | V_t]            (D, 768)   [precompute]
  Mk = [Wk.T @ [W_b | Winv.T@V_b | W_t] | X_t]    (D, 1024)  [precompute]
  A  = U.T @ Wo.T,  U = [U_b | U_t]               (512, D)   [precompute]
  qrank.T = Mq.T @ x.T          (768, tok)
  krank.T = Mk.T @ x.T          (1024, tok)
  cums    = scan(krank.T * m)   (1024, tok)   per-partition prefix scan + carry
  r1 = (qv*ckw + a_bi*qiw*civ) / nc
  r2 = a_tri * qvt*ckwt*zc / nc^2
  out.T = A.T @ [r1; r2].T      (D, tok)
Bias term (zero in this problem) is applied on the host if nonzero.
"""

import numpy as np

B, T, D, R = 4, 2048, 1024, 256
N_CORES = 8
TT = T // 2          # tokens per core
TB = 512             # token block (PSUM-bank sized)
NBLK = TT // TB
P = 128
DC = D // P          # 8 d-chunks

_PROGRAM_CACHE = {}
_LAST_IN_MAPS = None


def _build_program(mask_is_ones, alpha_bi, alpha_tri):
    import concourse.bass as bass
    import concourse.tile as tile
    import concourse.mybir as mybir
    from concourse import bacc
    from contextlib import ExitStack

    f32, f32r = mybir.dt.float32, mybir.dt.float32r
    ALU = mybir.AluOpType

    nc = bacc.Bacc(trn_type="TRN2")

    # ---- DRAM I/O ----
    xT = nc.dram_tensor("xT", [D, TT], f32r, kind="ExternalInput")
    initk = nc.dram_tensor("initk", [1024, 1], f32, kind="ExternalInput")
    invnc = nc.dram_tensor("invnc", [1, TT], f32, kind="ExternalInput")
    Wq = nc.dram_tensor("Wq", [D, D], f32r, kind="ExternalInput")
    Wk = nc.dram_tensor("Wk", [D, D], f32r, kind="ExternalInput")
    Winv = nc.dram_tensor("Winv", [D, D], f32r, kind="ExternalInput")
    WoT = nc.dram_tensor("WoT", [D, D], f32r, kind="ExternalInput")
    U = nc.dram_tensor("U", [D, 2 * R], f32r, kind="ExternalInput")
    FQ = nc.dram_tensor("FQ", [D, 2 * R], f32r, kind="ExternalInput")   # [V_b | V_t]
    FK = nc.dram_tensor("FK", [D, 2 * R], f32r, kind="ExternalInput")   # [W_b | W_t]
    Xt = nc.dram_tensor("Xt", [D, R], f32r, kind="ExternalInput")
    if not mask_is_ones:
        mrow = nc.dram_tensor("mrow", [1, TT], f32, kind="ExternalInput")
    outT = nc.dram_tensor("outT", [D, TT], f32, kind="ExternalOutput")

    def chunked(h):  # (D, N) dram -> (P, DC, N)
        return h.rearrange("(c p) n -> p c n", p=P)

    with tile.TileContext(nc) as tc:
        with ExitStack() as outer:
            # Resident factored weights (survive precompute scope)
            mpool = outer.enter_context(tc.tile_pool(name="m", bufs=1))
            mq_sb = mpool.tile([P, DC, 768], f32r)
            mk_sb = mpool.tile([P, DC, 1024], f32r)
            a_sb = mpool.tile([P, 4, D], f32r)

            # Xt goes straight into Mk columns 768:1024
            nc.sync.dma_start(mk_sb[:, :, 768:1024], chunked(Xt))

            # ================= precompute (part 1: N34, Mq, Mk) ==========
            with ExitStack() as pre:
                fpool = pre.enter_context(tc.tile_pool(name="fac", bufs=1))
                wpool = pre.enter_context(tc.tile_pool(name="wbig", bufs=2))
                pps = pre.enter_context(
                    tc.tile_pool(name="pps", bufs=2, space="PSUM"))

                fq_sb = fpool.tile([P, DC, 2 * R], f32r)
                nc.sync.dma_start(fq_sb[:], chunked(FQ))
                fk_sb = fpool.tile([P, DC, 2 * R], f32r)
                nc.sync.dma_start(fk_sb[:], chunked(FK))
                n34_sb = fpool.tile([P, DC, 2 * R], f32r)

                # N34 = Winv.T @ [W_b | V_b]
                wv = wpool.tile([P, DC, D], f32r, tag="wbig")
                nc.sync.dma_start(wv[:], chunked(Winv))
                for ic in range(DC):
                    ps = pps.tile([P, 512], f32)
                    isl = bass.ts(ic, P)
                    for oc in range(DC):
                        nc.tensor.matmul(ps[:, 0:256], wv[:, oc, isl],
                                         fk_sb[:, oc, 0:256],
                                         start=(oc == 0), stop=(oc == DC - 1))
                    for oc in range(DC):
                        nc.tensor.matmul(ps[:, 256:512], wv[:, oc, isl],
                                         fq_sb[:, oc, 0:256],
                                         start=(oc == 0), stop=(oc == DC - 1))
                    nc.vector.tensor_copy(n34_sb[:, ic, :], ps[:])

                # Mq = Wq.T @ [V_b | N3 | V_t], columns interleaved so that
                # chunk order is [qv0 qiw0 qvt0 qv1 qiw1 qvt1]
                wq = wpool.tile([P, DC, D], f32r, tag="wbig")
                nc.sync.dma_start(wq[:], chunked(Wq))
                for ic in range(DC):
                    ps = pps.tile([P, 768], f32)
                    isl = bass.ts(ic, P)
                    groups = [fq_sb[:, :, 0:256],      # V_b  -> qv
                              n34_sb[:, :, 0:256],     # N3   -> qiw
                              fq_sb[:, :, 256:512]]    # V_t  -> qvt
                    for g, rhs in enumerate(groups):
                        for oc in range(DC):
                            nc.tensor.matmul(ps[:, g * 256:(g + 1) * 256],
                                             wq[:, oc, isl], rhs[:, oc, :],
                                             start=(oc == 0), stop=(oc == DC - 1))
                    # reorder (g h c) -> (h g c): chunks become h-major
                    mq_dst = mq_sb[:, ic, :].rearrange("p (h q) -> p h q", h=2)
                    for g in range(3):
                        nc.vector.tensor_copy(
                            mq_dst[:, :, bass.ts(g, P)],
                            ps[:, bass.ts(g, 256)].rearrange(
                                "p (h c) -> p h c", h=2))

                # Mk[:, 0:768] = Wk.T @ [W_b | N4 | W_t]  (kept group-major)
                wk = wpool.tile([P, DC, D], f32r, tag="wbig")
                nc.sync.dma_start(wk[:], chunked(Wk))
                for ic in range(DC):
                    ps = pps.tile([P, 768], f32)
                    isl = bass.ts(ic, P)
                    groups = [fk_sb[:, :, 0:256],      # W_b -> ckw
                              n34_sb[:, :, 256:512],   # N4  -> civ
                              fk_sb[:, :, 256:512]]    # W_t -> ckwt
                    for g, rhs in enumerate(groups):
                        for oc in range(DC):
                            nc.tensor.matmul(ps[:, g * 256:(g + 1) * 256],
                                             wk[:, oc, isl], rhs[:, oc, :],
                                             start=(oc == 0), stop=(oc == DC - 1))
                    nc.vector.tensor_copy(mk_sb[:, ic, 0:768], ps[:])

            # ================= precompute (part 2: A) ====================
            with ExitStack() as pre:
                upool = pre.enter_context(tc.tile_pool(name="ures", bufs=1))
                wpool = pre.enter_context(tc.tile_pool(name="wbig2", bufs=1))
                pps = pre.enter_context(
                    tc.tile_pool(name="pps2", bufs=2, space="PSUM"))
                wo = wpool.tile([P, DC, D], f32r)
                nc.sync.dma_start(wo[:], chunked(WoT))
                u_sb = upool.tile([P, DC, 2 * R], f32r)
                nc.sync.dma_start(u_sb[:], chunked(U))
                for cc in range(4):
                    ps = pps.tile([P, D], f32)
                    csl = bass.ts(cc, P)
                    for half in range(2):
                        hsl = bass.ts(half, 512)
                        for ec in range(DC):
                            nc.tensor.matmul(ps[:, hsl], u_sb[:, ec, csl],
                                             wo[:, ec, hsl],
                                             start=(ec == 0), stop=(ec == DC - 1))
                    nc.vector.tensor_copy(a_sb[:, cc, :], ps[:])

            # ================= main loop =================
            with ExitStack() as mn:
                spool = mn.enter_context(tc.tile_pool(name="small", bufs=1))
                xpool = mn.enter_context(tc.tile_pool(name="xtb", bufs=2))
                cpool = mn.enter_context(tc.tile_pool(name="cc", bufs=2))
                rpool = mn.enter_context(tc.tile_pool(name="r", bufs=2))
                tpool = mn.enter_context(tc.tile_pool(name="tmp", bufs=6))
                opool = mn.enter_context(tc.tile_pool(name="osb", bufs=1))
                kps = mn.enter_context(
                    tc.tile_pool(name="kps", bufs=2, space="PSUM"))
                qps = mn.enter_context(
                    tc.tile_pool(name="qps", bufs=4, space="PSUM"))
                ops = mn.enter_context(
                    tc.tile_pool(name="ops", bufs=2, space="PSUM"))

                ini_sb = spool.tile([P, DC, 1], f32)
                nc.sync.dma_start(ini_sb[:],
                                  initk.rearrange("(c p) o -> p c o", p=P))
                inv_sb = spool.tile([P, TT], f32)
                nc.sync.dma_start(inv_sb[:], invnc[:].to_broadcast((P, TT)))
                if not mask_is_ones:
                    mb_sb = spool.tile([P, TT], f32)
                    nc.sync.dma_start(mb_sb[:], mrow[:].to_broadcast((P, TT)))

                xT_c = xT.rearrange("(c p) t -> p c t", p=P)
                outT_c = outT.rearrange("(c p) t -> p c t", p=P)

                xtbs, ccs, rs = [], [], []
                pend = []  # deferred output stages, one per block
                for j in range(NBLK):
                    tsl = bass.ds(j * TB, TB)
                    xtb = xpool.tile([P, DC, TB], f32r, tag="xtb")
                    nc.sync.dma_start(xtb[:], xT_c[:, :, tsl])
                    xtbs.append(xtb)

                    # ---- krank + scans ----
                    cc_t = cpool.tile([P, DC, TB], f32, tag="cc")
                    ccs.append(cc_t)
                    for kc in range(DC):
                        kr = kps.tile([P, TB], f32, tag="kr")
                        for dc in range(DC):
                            nc.tensor.matmul(kr[:], mk_sb[:, dc, bass.ts(kc, P)],
                                             xtb[:, dc, :],
                                             start=(dc == 0), stop=(dc == DC - 1))
                        ini = (ini_sb[:, kc, :] if j == 0
                               else ccs[j - 1][:, kc, TB - 1:TB])
                        if mask_is_ones:
                            nc.vector.tensor_tensor_scan(
                                cc_t[:, kc, :], kr[:], xtb[:, 0, :].bitcast(f32),
                                ini, op0=ALU.add, op1=ALU.bypass)
                        else:
                            km = tpool.tile([P, TB], f32, tag="tmp")
                            nc.vector.tensor_mul(km[:], kr[:], mb_sb[:, tsl])
                            nc.vector.tensor_tensor_scan(
                                cc_t[:, kc, :], km[:], km[:],
                                ini, op0=ALU.add, op1=ALU.bypass)

                    # ---- qrank + elementwise ----
                    r_t = rpool.tile([P, 4, TB], f32r, tag="r")
                    rs.append(r_t)
                    for h in range(2):
                        qtiles = []
                        for q3 in range(3):
                            qp = qps.tile([P, TB], f32, tag="qp")
                            qc = h * 3 + q3
                            for dc in range(DC):
                                nc.tensor.matmul(qp[:],
                                                 mq_sb[:, dc, bass.ts(qc, P)],
                                                 xtb[:, dc, :],
                                                 start=(dc == 0),
                                                 stop=(dc == DC - 1))
                            qtiles.append(qp)
                        qv, qiw, qvt = qtiles
                        ta = tpool.tile([P, TB], f32, tag="tmp")
                        nc.vector.tensor_mul(ta[:], qv[:], cc_t[:, 0 + h, :])
                        tb_ = tpool.tile([P, TB], f32, tag="tmp")
                        nc.vector.tensor_mul(tb_[:], qiw[:], cc_t[:, 2 + h, :])
                        tc_ = tpool.tile([P, TB], f32, tag="tmp")
                        nc.vector.scalar_tensor_tensor(
                            tc_[:], tb_[:], float(alpha_bi), ta[:],
                            op0=ALU.mult, op1=ALU.add)
                        nc.vector.tensor_mul(r_t[:, h, :], tc_[:], inv_sb[:, tsl])
                        td = tpool.tile([P, TB], f32, tag="tmp")
                        nc.vector.tensor_mul(td[:], qvt[:], cc_t[:, 4 + h, :])
                        te = tpool.tile([P, TB], f32, tag="tmp")
                        nc.vector.tensor_mul(te[:], td[:], cc_t[:, 6 + h, :])
                        te2 = tpool.tile([P, TB], f32, tag="tmp")
                        nc.vector.tensor_mul(te2[:], te[:], inv_sb[:, tsl])
                        nc.vector.scalar_tensor_tensor(
                            r_t[:, 2 + h, :], te2[:], float(alpha_tri),
                            inv_sb[:, tsl], op0=ALU.mult, op1=ALU.mult)

                    # ---- deferred output projection ----
                    def out_stage(jj):
                        tsl2 = bass.ds(jj * TB, TB)
                        osb = opool.tile([P, DC, TB], f32, tag="osb")
                        for dk in range(DC):
                            op = ops.tile([P, TB], f32, tag="op")
                            for cc4 in range(4):
                                nc.tensor.matmul(op[:],
                                                 a_sb[:, cc4, bass.ts(dk, P)],
                                                 rs[jj][:, cc4, :],
                                                 start=(cc4 == 0),
                                                 stop=(cc4 == 3))
                            nc.vector.tensor_copy(osb[:, dk, :], op[:])
                        nc.sync.dma_start(outT_c[:, :, tsl2], osb[:])

                    pend.append(out_stage)
                    if j > 0:
                        pend[j - 1](j - 1)
                pend[NBLK - 1](NBLK - 1)

    nc.finalize()
    return nc


def kernel(**inputs):
    from concourse.bass_utils import run_bass_kernel_spmd

    x = np.ascontiguousarray(np.asarray(inputs["x"], dtype=np.float32))
    mask = np.asarray(inputs["attention_mask"]).astype(np.float32)
    Wq = np.asarray(inputs["Wq"], np.float32)
    Wk = np.asarray(inputs["Wk"], np.float32)
    Wo = np.asarray(inputs["Wo"], np.float32)
    Winv = np.asarray(inputs["Winv"], np.float32)
    U_b = np.asarray(inputs["U_b"], np.float32)
    V_b = np.asarray(inputs["V_b"], np.float32)
    W_b = np.asarray(inputs["W_b"], np.float32)
    bias_b = np.asarray(inputs["bias_b"], np.float32)
    U_t = np.asarray(inputs["U_t"], np.float32)
    V_t = np.asarray(inputs["V_t"], np.float32)
    W_t = np.asarray(inputs["W_t"], np.float32)
    X_t = np.asarray(inputs["X_t"], np.float32)
    bias_t = np.asarray(inputs["bias_t"], np.float32)
    alpha_bi = float(np.asarray(inputs["alpha_bi"]))
    alpha_tri = float(np.asarray(inputs["alpha_tri"]))

    mask_is_ones = bool(mask.all())
    key = (mask_is_ones, alpha_bi, alpha_tri)
    if key not in _PROGRAM_CACHE:
        _PROGRAM_CACHE[key] = _build_program(mask_is_ones, alpha_bi, alpha_tri)
    nc = _PROGRAM_CACHE[key]

    # shared (replicated) weight arrays
    WoT = np.ascontiguousarray(Wo.T)
    U = np.ascontiguousarray(np.concatenate([U_b, U_t], axis=1))
    FQ = np.ascontiguousarray(np.concatenate([V_b, V_t], axis=1))
    FK = np.ascontiguousarray(np.concatenate([W_b, W_t], axis=1))
    shared = {"Wq": Wq, "Wk": Wk, "Winv": Winv, "WoT": WoT,
              "U": U, "FQ": FQ, "FK": FK, "Xt": np.ascontiguousarray(X_t)}

    in_maps = []
    for c in range(N_CORES):
        b, h = divmod(c, 2)
        sl = slice(h * TT, (h + 1) * TT)
        m_sh = mask[b, sl]
        xT_sh = np.ascontiguousarray(x[b, sl, :].T)
        # cross-shard carry: rank-space projection of masked first-half x sum
        if h == 0:
            S_prev = np.zeros(D, np.float32)
            n_prev = 0.0
        else:
            pm = mask[b, :TT]
            S_prev = (x[b, :TT, :] * pm[:, None]).sum(0).astype(np.float32)
            n_prev = float(pm.sum())
        Sk = S_prev @ Wk.T
        carry = np.concatenate([
            Sk @ W_b,
            (Sk @ Winv.T) @ V_b,
            Sk @ W_t,
            S_prev @ X_t,
        ]).astype(np.float32).reshape(1024, 1)
        n = n_prev + np.cumsum(m_sh)
        ncl = np.maximum(n, 1.0)
        inv = (1.0 / ncl).astype(np.float32).reshape(1, TT)
        im = dict(shared)
        im["xT"] = xT_sh
        im["initk"] = carry
        im["invnc"] = inv
        if not mask_is_ones:
            im["mrow"] = np.ascontiguousarray(m_sh.reshape(1, TT))
        in_maps.append(im)

    global _LAST_IN_MAPS
    _LAST_IN_MAPS = in_maps
    res = run_bass_kernel_spmd(nc, in_maps, core_ids=list(range(N_CORES)))

    out = np.empty((B, T, D), np.float32)
    for c in range(N_CORES):
        b, h = divmod(c, 2)
        out[b, h * TT:(h + 1) * TT, :] = res.results[c]["outT"].T

    # host-side bias term (zero for this problem's inputs)
    biasvec = bias_b * (1.0 + alpha_bi) + alpha_tri * bias_t
    if np.any(biasvec):
        biasproj = biasvec @ Wo.T
        n_full = np.cumsum(mask, axis=1)
        ratio = (n_full / np.maximum(n_full, 1.0)).astype(np.float32)
        out += ratio[:, :, None] * biasproj[None, None, :]
    return out
